# revision 13
# baseline (speedup 1.0000x reference)
"""RBF-kernel SVM decision function on 8 TRN2 NeuronCores (baseline v1).

out[i] = sum_j alphas[j] * exp(-GAMMA * ||x[i] - supports[j]||^2)

Hybrid ACT/DVE reduction; see kernel.py history. Known HW exec: ~146.9us.
"""

import os
import sys

for p in ("/opt/trn_rl_repo",):
    if p not in sys.path:
        sys.path.insert(0, p)

import numpy as np
import ml_dtypes

import concourse.bass as bass
import concourse.tile as tile
from concourse import bacc, mybir
from concourse.bass_utils import run_bass_kernel_spmd

N_CORES = 8
N = 16384
M = 8192
F = 64
GAMMA = 1.0 / F
N_LOC = N // N_CORES        # 2048 queries per core
N_TILES = N_LOC // 128      # 16 i-tiles of 128 queries
K_AUG = F + 2               # 66 contraction rows
W = 2048                    # j-window: 4 PSUM banks
NW = M // W                 # 4 windows per j sweep
MM_N = 512                  # matmul moving free dim (1 PSUM bank)
M_PAD = M + 256             # fp16 staging width (zero tail pad, mult of 4)

LN2 = float(np.log(1.0))  # placeholder, set below
LN2 = float(np.log(2.0))
S16 = 2.0**10 / LN2         # exponent pre-scale (fp16 bit-pattern units)
SIGMA = float(os.environ.get("BASS_SIGMA", "-0.0575"))
N_FAST = int(os.environ.get("BASS_FAST", "5"))  # fast-exp windows (DVE)
CA = 0.5
CB = 0.5 / float(np.sqrt(2.0))
I16 = None  # set after dtypes

BF16 = mybir.dt.bfloat16
FP16 = mybir.dt.float16
F32 = mybir.dt.float32
bf16 = ml_dtypes.bfloat16
I16 = mybir.dt.int16

_compiled_cache = {}


def _build_common(nc, tc, cpool):
    xaugT_d = nc.dram_tensor("xaugT", [K_AUG, N_LOC], BF16, kind="ExternalInput")
    saug_d = nc.dram_tensor("saug", [K_AUG, M], BF16, kind="ExternalInput")
    cbias_d = nc.dram_tensor("cbias", [128, N_TILES], F32, kind="ExternalInput")
    cbA_d = nc.dram_tensor("cbA", [128, N_TILES], F32, kind="ExternalInput")
    out_d = nc.dram_tensor("out", [128, N_TILES], F32, kind="ExternalOutput")

    warm_act = cpool.tile([128, 1], F32)
    nc.gpsimd.memset(warm_act[:], 0.0)
    nc.scalar.activation(warm_act[:], warm_act[:], mybir.ActivationFunctionType.Exp)

    saug_sb = cpool.tile([K_AUG, M], BF16)
    nc.sync.dma_start(saug_sb[:, 0:W], saug_d.ap()[:, 0:W])
    xaugT_sb = cpool.tile([K_AUG, N_LOC], BF16)
    nc.sync.dma_start(xaugT_sb[:, 0:128], xaugT_d.ap()[:, 0:128])
    cbias_sb = cpool.tile([128, N_TILES], F32)
    nc.sync.dma_start(cbias_sb[:], cbias_d.ap()[:])
    cbA_sb = cpool.tile([128, N_TILES], F32)
    nc.sync.dma_start(cbA_sb[:], cbA_d.ap()[:])
    for w in range(1, NW):
        nc.sync.dma_start(
            saug_sb[:, w * W : (w + 1) * W],
            saug_d.ap()[:, w * W : (w + 1) * W],
        )
    nc.sync.dma_start(xaugT_sb[:, 128:], xaugT_d.ap()[:, 128:])
    return xaugT_sb, saug_sb, cbias_sb, cbA_sb, out_d


def _mm_windows(nc, t, ps_tile, w, xaugT_sb, saug_sb):
    for c in range(W // MM_N):
        nc.tensor.matmul(
            ps_tile[:, c * MM_N : (c + 1) * MM_N],
            xaugT_sb[:, t * 128 : (t + 1) * 128],
            saug_sb[:, w * W + c * MM_N : w * W + (c + 1) * MM_N],
            start=True,
            stop=True,
        )


def _fast_set(n, b):
    """n fast-exp windows on even tiles (the all-staged tiles), never the
    boundary-straddling window, never the last tile."""
    w_mix = b // W
    wp = [w for w in (1, 3, 0, 2) if w != w_mix]
    out = set()
    for w in wp:
        for t in range(0, N_TILES - 1, 2):
            if len(out) >= n:
                return frozenset(out)
            out.add((t, w))
    return frozenset(out)


def _build_mix(b, fast_ws):
    nc = bacc.Bacc(
        "TRN2",
        target_bir_lowering=False,
        debug=False,
        enable_asserts=False,
        num_devices=N_CORES,
    )
    w_mix = b // W
    act_w = 0 if w_mix != 0 else 1

    def pieces_of(w):
        lo, hi = w * W, (w + 1) * W
        if b <= lo:
            return [(lo, hi, False)]
        if b >= hi:
            return [(lo, hi, True)]
        return [(lo, b, True), (b, hi, False)]

    def tile_sets(t):
        last = t == N_TILES - 1
        if last:
            act_set = set(range(NW))
        elif t % 2 == 0:
            act_set = set()
        else:
            act_set = {act_w}
        fast = {w for w in range(NW) if (t, w) in fast_ws}
        act_set -= fast
        return act_set, fast

    def tile_counts(t):
        act_set, fast = tile_sets(t)
        nP = nN = 0
        for w in range(NW):
            if w in fast:
                (_, _, pos), = pieces_of(w)
                if pos:
                    nP += 2
                else:
                    nN += 2
            else:
                for _, _, pos in pieces_of(w):
                    if pos:
                        nP += 1
                    else:
                        nN += 1
        return nP, nN

    with tile.TileContext(nc) as tc:
        with (
            tc.tile_pool(name="const", bufs=1) as cpool,
            tc.tile_pool(name="acc", bufs=3) as apool,
            tc.tile_pool(name="stg", bufs=3) as spool,
            tc.tile_pool(name="tree", bufs=3) as tpool,
            tc.tile_pool(name="psum", bufs=2, space="PSUM") as ppool,
        ):
            xaugT_sb, saug_sb, cbias_sb, cbA_sb, out_d = _build_common(nc, tc, cpool)
            outT_sb = cpool.tile([128, N_TILES], F32)
            dvout = cpool.tile([128, M], FP16)

            def tree_reduce(src_ap, wd, scalar, col):
                """fp16 pairwise tree (2x TT) + 512-max CACHE_REDUCE."""
                h = wd // 2
                th1 = tpool.tile([128, 1024], FP16, tag="th1")
                nc.vector.tensor_tensor(
                    th1[:, 0:h], src_ap[:, 0:h], src_ap[:, h:wd],
                    mybir.AluOpType.add,
                )
                q = h // 2
                th2 = tpool.tile([128, 512], FP16, tag="th2")
                nc.vector.tensor_tensor(
                    th2[:, 0:q], th1[:, 0:q], th1[:, q:h],
                    mybir.AluOpType.add,
                )
                nc.vector.tensor_scalar(
                    dvout[:, 0:q], th2[:, 0:q], scalar, 0.0,
                    mybir.AluOpType.mult, mybir.AluOpType.add,
                    accum_out=col,
                )

            for t in range(N_TILES):
                act_set, fast = tile_sets(t)
                nP, nN = tile_counts(t)
                accP = apool.tile([128, max(nP, 1)], F32, tag="accP")
                accN = apool.tile([128, max(nN, 1)], F32, tag="accN")
                iP = iN = 0

                def acc_col(pos):
                    nonlocal iP, iN
                    if pos:
                        col = accP[:, iP : iP + 1]
                        iP += 1
                    else:
                        col = accN[:, iN : iN + 1]
                        iN += 1
                    return col

                stg = spool.tile([128, M], FP16, tag="stg")
                for w in range(NW):
                    ps_tile = ppool.tile([128, W], F32, tag="E")
                    _mm_windows(nc, t, ps_tile, w, xaugT_sb, saug_sb)
                    if w in fast:
                        fstg = spool.tile([128, 2 * W], FP16, tag="fstg")
                        nc.vector.tensor_scalar(
                            fstg[:, 0:W].bitcast(I16),
                            ps_tile[:],
                            cbA_sb[:, t : t + 1],
                            0.0,
                            mybir.AluOpType.add,
                            mybir.AluOpType.max,
                        )
                        nc.vector.tensor_scalar(
                            fstg[:, W : 2 * W].bitcast(I16),
                            fstg[:, 0:W].bitcast(I16),
                            512.0,
                            None,
                            mybir.AluOpType.add,
                        )
                        (_, _, pos), = pieces_of(w)
                        for k, cph in ((0, CA), (1, CB)):
                            tree_reduce(
                                fstg[:, k * W : (k + 1) * W], W, cph, acc_col(pos)
                            )
                    elif w in act_set:
                        for lo, hi, pos in pieces_of(w):
                            nc.scalar.activation(
                                ps_tile[:, lo - w * W : hi - w * W],
                                ps_tile[:, lo - w * W : hi - w * W],
                                mybir.ActivationFunctionType.Exp,
                                bias=cbias_sb[:, t : t + 1],
                                scale=1.0 / S16,
                                accum_out=acc_col(pos),
                            )
                    else:
                        nc.scalar.activation(
                            stg[:, w * W : (w + 1) * W],
                            ps_tile[:],
                            mybir.ActivationFunctionType.Exp,
                            bias=cbias_sb[:, t : t + 1],
                            scale=1.0 / S16,
                        )
                for w in range(NW):
                    if w in act_set or w in fast:
                        continue
                    for lo, hi, pos in pieces_of(w):
                        wd = hi - lo
                        if wd % 4 == 0 and wd >= 512:
                            tree_reduce(stg[:, lo:hi], wd, 1.0, acc_col(pos))
                        else:
                            nc.vector.tensor_scalar(
                                dvout[:, lo:hi],
                                stg[:, lo:hi],
                                1.0,
                                0.0,
                                mybir.AluOpType.mult,
                                mybir.AluOpType.add,
                                accum_out=acc_col(pos),
                            )
                sumP = apool.tile([128, 1], F32, tag="sumP")
                nc.vector.reduce_sum(sumP[:], accP[:, :iP], axis=mybir.AxisListType.X)
                sumN = apool.tile([128, 1], F32, tag="sumN")
                nc.vector.reduce_sum(sumN[:], accN[:, :iN], axis=mybir.AxisListType.X)
                nc.vector.tensor_sub(outT_sb[:, t : t + 1], sumP[:], sumN[:])

            nc.sync.dma_start(out_d.ap()[:], outT_sb[:])

    nc.compile()
    return nc


def _prepare(x, supports, alphas):
    x = np.asarray(x, dtype=np.float32)
    supports = np.asarray(supports, dtype=np.float32)
    alphas = np.asarray(alphas, dtype=np.float32)

    a64 = alphas.astype(np.float64)
    s64 = supports.astype(np.float64)
    jterm = -GAMMA * (s64 * s64).sum(axis=1) + np.log(
        np.maximum(np.abs(a64), 1e-300)
    )
    jterm = np.maximum(jterm, -34.0)

    pos = a64 > 0
    perm = np.concatenate([np.nonzero(pos)[0], np.nonzero(~pos)[0]])
    b = int(pos.sum())

    jt = jterm[perm] * S16
    hi = jt.astype(bf16)
    lo = (jt - hi.astype(np.float64)).astype(bf16)

    saug = np.empty((K_AUG, M), dtype=bf16)
    saug[:F] = (
        supports[perm].T.astype(np.float64) * (2.0 * GAMMA * S16 * 32.0)
    ).astype(bf16)
    saug[F] = hi
    saug[F + 1] = lo

    xaugT = np.ones((K_AUG, N), dtype=bf16)
    xaugT[:F] = (x.T / 32.0).astype(bf16)

    ci = -GAMMA * (x.astype(np.float64) ** 2).sum(axis=1)
    cbias = ci.astype(np.float32)
    cbA = (ci * S16 + (15.0 + SIGMA) * 2.0**10).astype(np.float32)

    in_maps = []
    for c in range(N_CORES):
        sl = slice(c * N_LOC, (c + 1) * N_LOC)
        in_maps.append(
            {
                "xaugT": np.ascontiguousarray(xaugT[:, sl]),
                "saug": saug,
                "cbias": np.ascontiguousarray(
                    cbias[sl].reshape(N_TILES, 128).T
                ),
                "cbA": np.ascontiguousarray(
                    cbA[sl].reshape(N_TILES, 128).T
                ),
            }
        )
    return b, in_maps


def _run(x, supports, alphas, trace=False, **run_kwargs):
    b, in_maps = _prepare(x, supports, alphas)
    fast_ws = _fast_set(N_FAST, b)
    key = (b, fast_ws, SIGMA)
    if key not in _compiled_cache:
        _compiled_cache[key] = _build_mix(b, fast_ws)
    nc = _compiled_cache[key]
    res = run_bass_kernel_spmd(
        nc, in_maps, core_ids=list(range(N_CORES)), trace=trace, **run_kwargs
    )
    outs = [r["out"].T.reshape(-1) for r in res.results]
    return np.concatenate(outs).astype(np.float32), res


def kernel(x, supports, alphas):
    out, _ = _run(x, supports, alphas, trace=False)
    return out


# revision 14
# speedup vs baseline: 1.0016x; 1.0016x over previous
"""RBF-kernel SVM decision function on 8 TRN2 NeuronCores (baseline v1).

out[i] = sum_j alphas[j] * exp(-GAMMA * ||x[i] - supports[j]||^2)

Hybrid ACT/DVE reduction; see kernel.py history. Known HW exec: ~146.9us.
"""

import os
import sys

for p in ("/opt/trn_rl_repo",):
    if p not in sys.path:
        sys.path.insert(0, p)

import numpy as np
import ml_dtypes

import concourse.bass as bass
import concourse.tile as tile
from concourse import bacc, mybir
from concourse.bass_utils import run_bass_kernel_spmd

N_CORES = 8
N = 16384
M = 8192
F = 64
GAMMA = 1.0 / F
N_LOC = N // N_CORES        # 2048 queries per core
N_TILES = N_LOC // 128      # 16 i-tiles of 128 queries
K_AUG = F + 2               # 66 contraction rows
W = 2048                    # j-window: 4 PSUM banks
NW = M // W                 # 4 windows per j sweep
MM_N = 512                  # matmul moving free dim (1 PSUM bank)
M_PAD = M + 256             # fp16 staging width (zero tail pad, mult of 4)

LN2 = float(np.log(1.0))  # placeholder, set below
LN2 = float(np.log(2.0))
S16 = 2.0**10 / LN2         # exponent pre-scale (fp16 bit-pattern units)
SIGMA = float(os.environ.get("BASS_SIGMA", "-0.0575"))
N_FAST = int(os.environ.get("BASS_FAST", "5"))  # fast-exp windows (DVE)
CA = 0.5
CB = 0.5 / float(np.sqrt(2.0))
I16 = None  # set after dtypes

BF16 = mybir.dt.bfloat16
FP16 = mybir.dt.float16
F32 = mybir.dt.float32
bf16 = ml_dtypes.bfloat16
I16 = mybir.dt.int16

_compiled_cache = {}


def _build_common(nc, tc, cpool):
    xaugT_d = nc.dram_tensor("xaugT", [K_AUG, N_LOC], BF16, kind="ExternalInput")
    saug_d = nc.dram_tensor("saug", [K_AUG, M], BF16, kind="ExternalInput")
    cbias_d = nc.dram_tensor("cbias", [128, N_TILES], F32, kind="ExternalInput")
    cbA_d = nc.dram_tensor("cbA", [128, N_TILES], F32, kind="ExternalInput")
    out_d = nc.dram_tensor("out", [128, N_TILES], F32, kind="ExternalOutput")

    warm_act = cpool.tile([128, 1], F32)
    nc.gpsimd.memset(warm_act[:], 0.0)
    nc.scalar.activation(warm_act[:], warm_act[:], mybir.ActivationFunctionType.Exp)

    saug_sb = cpool.tile([K_AUG, M], BF16)
    nc.sync.dma_start(saug_sb[:, 0:W], saug_d.ap()[:, 0:W])
    xaugT_sb = cpool.tile([K_AUG, N_LOC], BF16)
    nc.sync.dma_start(xaugT_sb[:, 0:128], xaugT_d.ap()[:, 0:128])
    cbias_sb = cpool.tile([128, N_TILES], F32)
    nc.sync.dma_start(cbias_sb[:], cbias_d.ap()[:])
    cbA_sb = cpool.tile([128, N_TILES], F32)
    nc.sync.dma_start(cbA_sb[:], cbA_d.ap()[:])
    for w in range(1, NW):
        nc.sync.dma_start(
            saug_sb[:, w * W : (w + 1) * W],
            saug_d.ap()[:, w * W : (w + 1) * W],
        )
    nc.sync.dma_start(xaugT_sb[:, 128:], xaugT_d.ap()[:, 128:])
    return xaugT_sb, saug_sb, cbias_sb, cbA_sb, out_d


def _mm_windows(nc, t, ps_tile, w, xaugT_sb, saug_sb):
    for c in range(W // MM_N):
        nc.tensor.matmul(
            ps_tile[:, c * MM_N : (c + 1) * MM_N],
            xaugT_sb[:, t * 128 : (t + 1) * 128],
            saug_sb[:, w * W + c * MM_N : w * W + (c + 1) * MM_N],
            start=True,
            stop=True,
        )


def _fast_set(n, b):
    """n fast-exp windows on even tiles (the all-staged tiles), never the
    boundary-straddling window, never the last tile."""
    w_mix = b // W
    wp = [w for w in (1, 3, 0, 2) if w != w_mix]
    out = set()
    for w in wp:
        for t in range(0, N_TILES - 1, 2):
            if len(out) >= n:
                return frozenset(out)
            out.add((t, w))
    return frozenset(out)


def _build_mix(b, fast_ws):
    nc = bacc.Bacc(
        "TRN2",
        target_bir_lowering=False,
        debug=False,
        enable_asserts=False,
        num_devices=N_CORES,
    )
    w_mix = b // W
    act_w = 0 if w_mix != 0 else 1

    def pieces_of(w):
        lo, hi = w * W, (w + 1) * W
        if b <= lo:
            return [(lo, hi, False)]
        if b >= hi:
            return [(lo, hi, True)]
        return [(lo, b, True), (b, hi, False)]

    def tile_sets(t):
        last = t == N_TILES - 1
        if last:
            act_set = set(range(NW))
        elif t % 2 == 0:
            act_set = set()
        else:
            act_set = {act_w}
        fast = {w for w in range(NW) if (t, w) in fast_ws}
        act_set -= fast
        return act_set, fast

    def tile_counts(t):
        act_set, fast = tile_sets(t)
        nP = nN = 0
        for w in range(NW):
            if w in fast:
                (_, _, pos), = pieces_of(w)
                if pos:
                    nP += 2
                else:
                    nN += 2
            else:
                for _, _, pos in pieces_of(w):
                    if pos:
                        nP += 1
                    else:
                        nN += 1
        return nP, nN

    with tile.TileContext(nc) as tc:
        with (
            tc.tile_pool(name="const", bufs=1) as cpool,
            tc.tile_pool(name="acc", bufs=6) as apool,
            tc.tile_pool(name="stg", bufs=3) as spool,
            tc.tile_pool(name="tree", bufs=3) as tpool,
            tc.tile_pool(name="psum", bufs=2, space="PSUM") as ppool,
        ):
            xaugT_sb, saug_sb, cbias_sb, cbA_sb, out_d = _build_common(nc, tc, cpool)
            outT_sb = cpool.tile([128, N_TILES], F32)
            dvout = cpool.tile([128, M], FP16)
            pending = []  # deferred per-tile finishers (decouple DVE FIFO)

            def tree_reduce(src_ap, wd, scalar, col):
                """fp16 pairwise tree (2x TT) + 512-max CACHE_REDUCE."""
                h = wd // 2
                th1 = tpool.tile([128, 1024], FP16, tag="th1")
                nc.vector.tensor_tensor(
                    th1[:, 0:h], src_ap[:, 0:h], src_ap[:, h:wd],
                    mybir.AluOpType.add,
                )
                q = h // 2
                th2 = tpool.tile([128, 512], FP16, tag="th2")
                nc.vector.tensor_tensor(
                    th2[:, 0:q], th1[:, 0:q], th1[:, q:h],
                    mybir.AluOpType.add,
                )
                nc.vector.tensor_scalar(
                    dvout[:, 0:q], th2[:, 0:q], scalar, 0.0,
                    mybir.AluOpType.mult, mybir.AluOpType.add,
                    accum_out=col,
                )

            for t in range(N_TILES):
                act_set, fast = tile_sets(t)
                nP, nN = tile_counts(t)
                accP = apool.tile([128, max(nP, 1)], F32, tag="accP")
                accN = apool.tile([128, max(nN, 1)], F32, tag="accN")
                iP = iN = 0

                def acc_col(pos):
                    nonlocal iP, iN
                    if pos:
                        col = accP[:, iP : iP + 1]
                        iP += 1
                    else:
                        col = accN[:, iN : iN + 1]
                        iN += 1
                    return col

                stg = spool.tile([128, M], FP16, tag="stg")
                for w in range(NW):
                    ps_tile = ppool.tile([128, W], F32, tag="E")
                    _mm_windows(nc, t, ps_tile, w, xaugT_sb, saug_sb)
                    if w in fast:
                        fstg = spool.tile([128, 2 * W], FP16, tag="fstg")
                        nc.vector.tensor_scalar(
                            fstg[:, 0:W].bitcast(I16),
                            ps_tile[:],
                            cbA_sb[:, t : t + 1],
                            0.0,
                            mybir.AluOpType.add,
                            mybir.AluOpType.max,
                        )
                        nc.vector.tensor_scalar(
                            fstg[:, W : 2 * W].bitcast(I16),
                            fstg[:, 0:W].bitcast(I16),
                            512.0,
                            None,
                            mybir.AluOpType.add,
                        )
                        (_, _, pos), = pieces_of(w)
                        for k, cph in ((0, CA), (1, CB)):
                            tree_reduce(
                                fstg[:, k * W : (k + 1) * W], W, cph, acc_col(pos)
                            )
                    elif w in act_set:
                        for lo, hi, pos in pieces_of(w):
                            nc.scalar.activation(
                                ps_tile[:, lo - w * W : hi - w * W],
                                ps_tile[:, lo - w * W : hi - w * W],
                                mybir.ActivationFunctionType.Exp,
                                bias=cbias_sb[:, t : t + 1],
                                scale=1.0 / S16,
                                accum_out=acc_col(pos),
                            )
                    else:
                        nc.scalar.activation(
                            stg[:, w * W : (w + 1) * W],
                            ps_tile[:],
                            mybir.ActivationFunctionType.Exp,
                            bias=cbias_sb[:, t : t + 1],
                            scale=1.0 / S16,
                        )
                for w in range(NW):
                    if w in act_set or w in fast:
                        continue
                    for lo, hi, pos in pieces_of(w):
                        wd = hi - lo
                        if wd % 4 == 0 and wd >= 512:
                            tree_reduce(stg[:, lo:hi], wd, 1.0, acc_col(pos))
                        else:
                            nc.vector.tensor_scalar(
                                dvout[:, lo:hi],
                                stg[:, lo:hi],
                                1.0,
                                0.0,
                                mybir.AluOpType.mult,
                                mybir.AluOpType.add,
                                accum_out=acc_col(pos),
                            )
                def finisher(t=t, accP=accP, accN=accN, iP=iP, iN=iN):
                    sumP = apool.tile([128, 1], F32, tag="sumP")
                    nc.vector.reduce_sum(
                        sumP[:], accP[:, :iP], axis=mybir.AxisListType.X
                    )
                    sumN = apool.tile([128, 1], F32, tag="sumN")
                    nc.vector.reduce_sum(
                        sumN[:], accN[:, :iN], axis=mybir.AxisListType.X
                    )
                    nc.vector.tensor_sub(outT_sb[:, t : t + 1], sumP[:], sumN[:])

                pending.append(finisher)
                if len(pending) > 2:
                    pending.pop(0)()

            for fin in pending:
                fin()

            nc.sync.dma_start(out_d.ap()[:], outT_sb[:])

    nc.compile()
    return nc


def _prepare(x, supports, alphas):
    x = np.asarray(x, dtype=np.float32)
    supports = np.asarray(supports, dtype=np.float32)
    alphas = np.asarray(alphas, dtype=np.float32)

    a64 = alphas.astype(np.float64)
    s64 = supports.astype(np.float64)
    jterm = -GAMMA * (s64 * s64).sum(axis=1) + np.log(
        np.maximum(np.abs(a64), 1e-300)
    )
    jterm = np.maximum(jterm, -34.0)

    pos = a64 > 0
    perm = np.concatenate([np.nonzero(pos)[0], np.nonzero(~pos)[0]])
    b = int(pos.sum())

    jt = jterm[perm] * S16
    hi = jt.astype(bf16)
    lo = (jt - hi.astype(np.float64)).astype(bf16)

    saug = np.empty((K_AUG, M), dtype=bf16)
    saug[:F] = (
        supports[perm].T.astype(np.float64) * (2.0 * GAMMA * S16 * 32.0)
    ).astype(bf16)
    saug[F] = hi
    saug[F + 1] = lo

    xaugT = np.ones((K_AUG, N), dtype=bf16)
    xaugT[:F] = (x.T / 32.0).astype(bf16)

    ci = -GAMMA * (x.astype(np.float64) ** 2).sum(axis=1)
    cbias = ci.astype(np.float32)
    cbA = (ci * S16 + (15.0 + SIGMA) * 2.0**10).astype(np.float32)

    in_maps = []
    for c in range(N_CORES):
        sl = slice(c * N_LOC, (c + 1) * N_LOC)
        in_maps.append(
            {
                "xaugT": np.ascontiguousarray(xaugT[:, sl]),
                "saug": saug,
                "cbias": np.ascontiguousarray(
                    cbias[sl].reshape(N_TILES, 128).T
                ),
                "cbA": np.ascontiguousarray(
                    cbA[sl].reshape(N_TILES, 128).T
                ),
            }
        )
    return b, in_maps


def _run(x, supports, alphas, trace=False, **run_kwargs):
    b, in_maps = _prepare(x, supports, alphas)
    fast_ws = _fast_set(N_FAST, b)
    key = (b, fast_ws, SIGMA)
    if key not in _compiled_cache:
        _compiled_cache[key] = _build_mix(b, fast_ws)
    nc = _compiled_cache[key]
    res = run_bass_kernel_spmd(
        nc, in_maps, core_ids=list(range(N_CORES)), trace=trace, **run_kwargs
    )
    outs = [r["out"].T.reshape(-1) for r in res.results]
    return np.concatenate(outs).astype(np.float32), res


def kernel(x, supports, alphas):
    out, _ = _run(x, supports, alphas, trace=False)
    return out
